# revision 1
# baseline (speedup 1.0000x reference)
"""Additive attention (nn_AdditiveAttention) Trainium2 Bass kernel.

Math (per batch b):
    qp = queries @ W_q.T            # [Q, H]
    kp = keys    @ W_k.T            # [K, H]
    scores[q,k] = sum_h w_v[h] * tanh(qp[q,h] + kp[k,h])
    attn = softmax(scores, axis=-1)
    out  = attn @ values            # [Q, H]

Sharding: B*Q = 8 shards of 256 queries -> core c handles batch c//2,
query half c%2 (full K for that batch). Pure data parallel, no collectives.

Per-core plan (fp32 projections/softmax/attn@V, fp16 tanh pipeline):
  - Host pre-transposes qT=[H,Qc], kT=[H,K], WqT, WkT so the on-chip
    projections contract over partitions without any device transposes.
  - Projections on PE (fp32) -> qp (fp32) / kp (fp16) stored
    [h-part, q/k-free] in SBUF.
  - Main loop over (hc in 2, q): DVE tensor_scalar_add builds
    S[h=128, k=512] = kp[hc] + qp[hc][:, q] in fp16 (4x DVE mode,
    ~257ns), grouped NQ=16 queries per tile; ACT does one [128, 8192]
    tanh per group (~6.7us, the bottleneck engine: 1 elem/lane/cycle
    @1.2GHz, dtype-independent -> ~214us/core floor); PE reduces over h
    with an fp16 matvec (fp32 rhs would be 4x slower on the PE) whose
    w_v column is embedded at column jj of a [128,32] zero-padded lhsT
    so the M=32 output lands on the right PSUM rows (matmul output base
    partition must be 0/32/64/96). Query qb maps to PSUM row
    (qb%4)*32 + qb//4 so consecutive matvecs cycle the four 32-col
    PE array groups and overlap; rows are un-permuted by the output
    DMA access pattern.
  - Softmax per 128-q block: reduce_max(negate) -> exp(bias=-max) with
    fused accum_out row-sum -> reciprocal; applied to the attn@V output.
  - attn@V: PE transpose of attn (4x128 blocks) then 4 accumulating
    fp32 matmuls against values in natural [k,h] layout.

Engine budgets per core (HW-calibrated): ACT ~34 tanh/exp insts
~= 215us busy (bottleneck; 1 elem/lane/cycle, dtype-independent),
DVE 512 adds + epilogue ~= 135us, PE ~110us warm, DMA ~1.3MB.
Group sizes taper at the kernel head ([4,4,8,16...]) and final block
([8]*15+[4,4]) to shorten the fill ramp and the post-tanh PE tail;
projection inputs are fp16 (4x faster PE streaming than fp32).
Measured end-to-end ~175us/core (repeat-NEFF quantile-delta method,
+-50us measurement noise); cost-model timeline predicts 255us with
ACT 88% utilized.
"""

import numpy as np


def _split_multi_waits(nc, mybir):
    """walrus in this env rejects >1 sem wait per instruction; hoist extras
    onto same-engine NoOps inserted right before the instruction."""
    n_split = 0
    for bb in nc.m.functions[0].blocks:
        insts = bb.instructions
        i = 0
        while i < len(insts):
            ins = insts[i]
            si = ins.sync_info
            if si is not None and si.on_wait and len(si.on_wait) > 1:
                waits = list(si.on_wait)
                for w in waits[:-1]:
                    nop = mybir.InstNoOp(name=f"I-{nc.next_id()}", ins=[], outs=[])
                    nop.engine = ins.engine
                    nop.sync_info = mybir.SyncInfo(on_wait=[w], on_update=[])
                    nc.register_instruction(nop)
                    insts.insert(i, nop)
                    i += 1
                    n_split += 1
                si.on_wait = [waits[-1]]
            i += 1
    return n_split


B, Q, K, H = 4, 512, 512, 256
N_CORES = 8
QC = B * Q // N_CORES  # 256 queries per core
NQ = 16  # queries per tanh group
_BUILT = {}


def _build(repeats=1, mode="full"):
    import concourse.bass as bass
    import concourse.tile as tile
    from concourse import mybir
    from concourse.masks import make_identity

    f32 = mybir.dt.float32
    f16d = mybir.dt.float16
    f16 = mybir.dt.float16  # same DVE/PE rates as bf16, 8 more mantissa bits
    nc = bass.Bass()

    qT_d = nc.dram_tensor("qT", [H, QC], f16d, kind="ExternalInput")
    kT_d = nc.dram_tensor("kT", [H, K], f16d, kind="ExternalInput")
    wqT_d = nc.dram_tensor("wqT", [H, H], f16d, kind="ExternalInput")
    wkT_d = nc.dram_tensor("wkT", [H, H], f16d, kind="ExternalInput")
    wv_d = nc.dram_tensor("wv", [128, 2], f32, kind="ExternalInput")
    vals_d = nc.dram_tensor("vals", [K, H], f32, kind="ExternalInput")
    out_d = nc.dram_tensor("out", [QC, H], f32, kind="ExternalOutput")

    TANH = mybir.ActivationFunctionType.Tanh
    EXP = mybir.ActivationFunctionType.Exp
    X = mybir.AxisListType.X

    with tile.TileContext(nc) as tc:
        with (
            tc.tile_pool(name="const", bufs=1) as const,
            tc.tile_pool(name="spool", bufs=2) as spool,
            tc.tile_pool(name="fpool", bufs=6) as fpool,
            tc.tile_pool(name="work", bufs=2) as work,
            tc.tile_pool(name="stats", bufs=4) as stats,
            tc.tile_pool(name="ps_scores", bufs=2, space="PSUM") as ps_scores,
            tc.tile_pool(name="ps_mix", bufs=2, space="PSUM") as ps_mix,
            tc.tile_pool(name="ps_tr", bufs=2, space="PSUM") as ps_tr,
        ):
            # ---- static loads (kp-projection inputs first: critical path;
            # per-c chunks so the c=0 projection matmuls start at half-load)
            wkT_sb = const.tile([128, 2, H], f16d, tag="wkT_sb")
            kT_sb = const.tile([128, 2, K], f16d, tag="kT_sb")
            wqT_sb = const.tile([128, 2, H], f16d, tag="wqT_sb")
            qT_sb = const.tile([128, 2, QC], f16d, tag="qT_sb")
            for c in range(2):
                nc.sync.dma_start(
                    wkT_sb[:, c], wkT_d.rearrange("(c p) h -> c p h", p=128)[c]
                )
                nc.sync.dma_start(
                    kT_sb[:, c], kT_d.rearrange("(c p) k -> c p k", p=128)[c]
                )
                nc.sync.dma_start(
                    wqT_sb[:, c], wqT_d.rearrange("(c p) h -> c p h", p=128)[c]
                )
                nc.sync.dma_start(
                    qT_sb[:, c], qT_d.rearrange("(c p) q -> c p q", p=128)[c]
                )
            wv_sb = const.tile([128, 2], f32, tag="wv_sb")
            nc.sync.dma_start(wv_sb, wv_d[:, :])
            vals_sb = const.tile([128, 4, H], f32, tag="vals_sb")
            nc.sync.dma_start(vals_sb, vals_d.rearrange("(c p) h -> p c h", p=128))

            ident = const.tile([128, 128], f32, tag="ident")
            make_identity(nc, ident)

            # prime the ACT tanh/exp table set off the critical path: the
            # first ACTIVATE triggers a ~2.7us table DMA; do it on a dummy
            # element at t~0 so the first real tanh doesn't pay it
            primer = const.tile([128, 1], f32, tag="primer")
            nc.vector.memset(primer, 0.0)
            nc.scalar.activation(primer, primer, TANH)

            # i32rep[p, j, c] = (j == c) ? 1 : 0, independent of partition
            i32rep = const.tile([128, 32, 32], f32, tag="i32rep")
            nc.gpsimd.memset(i32rep, 0.0)
            nc.gpsimd.affine_select(
                out=i32rep,
                in_=i32rep,
                compare_op=mybir.AluOpType.not_equal,
                fill=1.0,
                base=0,
                pattern=[[1, 32], [-1, 32]],
                channel_multiplier=0,
            )
            # wv_emb[:, hc, j, c] = (j == c) * w_v[hc*128 + p]
            wv_emb = const.tile([128, 2, 32, 32], f16, tag="wv_emb")
            for hc in range(2):
                nc.gpsimd.tensor_scalar_mul(
                    wv_emb[:, hc], i32rep, wv_sb[:, hc : hc + 1]
                )

            for _rep in range(repeats):
                # ---- projections --------------------------------------------
                # qp[h, q] = sum_h' W_q[h, h'] * queries[q, h']
                qp_sb = const.tile([128, 2, QC], f32, tag="qp_sb")
                kp_sb = const.tile([128, 2, K], f16, tag="kp_sb")
                for hc in range(2):
                    # kp[hc] + qp[hc] together: the first tanh group only
                    # needs the hc=0 pair, so it unlocks 2 matmuls earlier
                    kp_ps = ps_mix.tile([128, K], f32, tag="mix", name="kp_ps")
                    for c in range(2):
                        nc.tensor.matmul(
                            kp_ps,
                            lhsT=wkT_sb[:, c, hc * 128 : (hc + 1) * 128],
                            rhs=kT_sb[:, c, :],
                            start=(c == 0),
                            stop=(c == 1),
                        )
                    nc.vector.tensor_copy(kp_sb[:, hc, :], kp_ps)
                    qp_ps = ps_mix.tile([128, K], f32, tag="mix", name="qp_ps")
                    for c in range(2):
                        nc.tensor.matmul(
                            qp_ps[:, :QC],
                            lhsT=wqT_sb[:, c, hc * 128 : (hc + 1) * 128],
                            rhs=qT_sb[:, c, :],
                            start=(c == 0),
                            stop=(c == 1),
                        )
                    nc.vector.tensor_copy(qp_sb[:, hc, :], qp_ps[:, :QC])

                # ---- main loop ----------------------------------------------
                # Query qb (position in 128-q block) accumulates its scores in
                # PSUM row perm(qb) = (qb%4)*32 + qb//4, so consecutive
                # matmuls cycle through the four 32-partition col-groups of
                # the PE array and overlap ~4-way (tile_position col tiling).
                # The final output DMA un-permutes rows via its access
                # pattern.
                n_blk = QC // 128
                for blk in range(n_blk):
                    # taper group sizes at the kernel head (shorter fill ramp
                    # before the first tanh) and tail (smaller final matvec
                    # batch after the last tanh)
                    if blk == 0:
                        sched = [4, 4, 8] + [NQ] * ((112) // NQ)
                    elif blk == n_blk - 1:
                        sched = [8] * 15 + [4, 4]
                    else:
                        sched = [NQ] * (128 // NQ)
                    scores = ps_scores.tile([128, K], f32, tag="scores", name="scores")
                    qstart = 0
                    for nq in sched:
                        g0 = qstart
                        qstart += nq
                        Fs = []
                        for hc in range(2):
                            S = spool.tile([128, NQ, K], f16, tag="S", name="S")
                            for j in range(nq):
                                q = blk * 128 + g0 + j
                                nc.vector.tensor_scalar_add(
                                    S[:, j, :], kp_sb[:, hc, :], qp_sb[:, hc, q : q + 1]
                                )
                            F = fpool.tile([128, NQ, K], f16, tag="F", name="F")
                            nc.scalar.activation(F[:, :nq], S[:, :nq], TANH)
                            Fs.append(F)
                        for j in range(nq if mode != "noscore" else 0):
                            qb = g0 + j
                            grp, jj = qb % 4, qb // 4
                            for hc in range(2):
                                nc.tensor.matmul(
                                    scores[grp * 32 : (grp + 1) * 32, :],
                                    lhsT=wv_emb[:, hc, jj, :],
                                    rhs=Fs[hc][:, j, :],
                                    start=(jj == 0 and hc == 0),
                                    stop=(jj == 31 and hc == 1),
                                    tile_position=(0, 32 * grp),
                                    skip_group_check=True,
                                )

                    if mode in ("noscore", "nosm"):
                        dumm = work.tile([128, H], f32, tag="ob", name="dumm")
                        nc.vector.tensor_copy(dumm, Fs[0][:, 0, 0:H])
                        nc.sync.dma_start(
                            out_d[blk * 128 : (blk + 1) * 128, :], dumm
                        )
                        continue
                    # softmax over k (rows are queries)
                    neg_mx = stats.tile([128, 1], f32, tag="neg_mx", name="neg_mx")
                    nc.vector.tensor_reduce(
                        neg_mx, scores, axis=X, op=mybir.AluOpType.max, negate=True
                    )
                    attn = work.tile([128, K], f32, tag="attn", name="attn")
                    sumexp = stats.tile([128, 1], f32, tag="sumexp", name="sumexp")
                    nc.scalar.activation(
                        attn, scores, EXP, bias=neg_mx, scale=1.0, accum_out=sumexp
                    )
                    rec = stats.tile([128, 1], f32, tag="rec", name="rec")
                    nc.vector.reciprocal(rec, sumexp)

                    # attn.T blocks then attn @ V
                    attnT = work.tile([128, 4, 128], f32, tag="attnT", name="attnT")
                    for kc in range(4):
                        tp = ps_tr.tile([128, 128], f32, tag="tp", name="tp")
                        nc.tensor.transpose(
                            tp, attn[:, kc * 128 : (kc + 1) * 128], ident
                        )
                        nc.vector.tensor_copy(attnT[:, kc, :], tp)
                    o_ps = ps_mix.tile([128, K], f32, tag="mix", name="o_ps")
                    for kc in range(4):
                        nc.tensor.matmul(
                            o_ps[:, :H],
                            lhsT=attnT[:, kc, :],
                            rhs=vals_sb[:, kc, :],
                            start=(kc == 0),
                            stop=(kc == 3),
                        )
                    ob = work.tile([128, H], f32, tag="ob", name="ob")
                    nc.vector.tensor_scalar_mul(ob, o_ps[:, :H], rec)
                    # partition r = 32a+b holds query 4b+a; un-permute rows
                    nc.sync.dma_start(
                        out_d[blk * 128 : (blk + 1) * 128, :].rearrange(
                            "(b a) h -> a b h", a=4
                        ),
                        ob,
                    )

    _split_multi_waits(nc, mybir)
    return nc


def _get_nc(repeats=1, mode="full"):
    key = f"nc{repeats}:{mode}"
    if key not in _BUILT:
        _BUILT[key] = _build(repeats, mode)
    return _BUILT[key]


def _in_maps(queries, keys, values, W_q, W_k, w_v):
    queries = np.asarray(queries, dtype=np.float32)
    keys = np.asarray(keys, dtype=np.float32)
    values = np.asarray(values, dtype=np.float32)
    W_q = np.asarray(W_q, dtype=np.float32)
    W_k = np.asarray(W_k, dtype=np.float32)
    w_v = np.asarray(w_v, dtype=np.float32)

    wqT = np.ascontiguousarray(W_q.T, dtype=np.float16)
    wkT = np.ascontiguousarray(W_k.T, dtype=np.float16)
    wv2 = np.ascontiguousarray(w_v.reshape(2, 128).T)
    maps = []
    for core in range(N_CORES):
        b, half = divmod(core, 2)
        qsl = queries[b, half * QC : (half + 1) * QC, :]
        maps.append(
            {
                "qT": np.ascontiguousarray(qsl.T, dtype=np.float16),
                "kT": np.ascontiguousarray(keys[b].T, dtype=np.float16),
                "wqT": wqT,
                "wkT": wkT,
                "wv": wv2,
                "vals": np.ascontiguousarray(values[b]),
            }
        )
    return maps


def kernel(queries, keys, values, W_q, W_k, w_v):
    from concourse.bass_utils import run_bass_kernel_spmd

    nc = _get_nc()
    maps = _in_maps(queries, keys, values, W_q, W_k, w_v)
    res = run_bass_kernel_spmd(nc, maps, core_ids=list(range(N_CORES)))
    out = np.empty((B, Q, H), np.float32)
    for core in range(N_CORES):
        b, half = divmod(core, 2)
        out[b, half * QC : (half + 1) * QC, :] = res.results[core]["out"]
    return out

